# revision 13
# baseline (speedup 1.0000x reference)
"""Multi-head attention (B=2, S=2048, D=1024, H=16) on 8 TRN2 NeuronCores.

Sharding: core = (batch b, head-group g): 2 batches x 4 groups of 4 heads.
Each core computes its group's QKV projections, attention, and a partial
output projection; the host sums the 4 partials per batch and adds the
exact bias constant (bv @ Wo.T + bo). bq/bk are applied on device.

v2 schedule (derived from the v1 trace: ACT exp 150us was the bottleneck
and only started at 75us because xq/wq landed last on the DMA queues; the
tail serialized 44us of evac->recip->broadcast->norm->out-proj):

  * all host-side tensors are prepacked into [128, N] row-major layouts so
    every DMA is a plain contiguous 2D copy (the v1 512B-elem rearranged
    weight DMAs ran at ~56GB/s and stalled the first projection).
  * two HWDGE queues (sync + gpsimd) stream inputs in consumption order:
    K then Q inputs first (9MB -> exp starts ~30us), xv/wv behind them,
    wo last. The scalar queue carries no DMA so ACT only runs exp.
  * K and Q projections are j-split (4 PSUM banks each) so the kt/qt
    evacuations pipeline with the next projection's matmuls.
  * attention runs in 8 units (q-half e outer, head h inner), each unit =
    16 key-tiles: scores -> exp[128,1024] -> PV into a double-buffered
    half-otp [128,1024]. The scores stream runs one unit ahead of the PV
    stream. PSUM: sp 2x2 banks + otp 2x2 banks = 8.
  * the V projection has no PSUM of its own: its [128,jj] tiles rotate
    through the scores pool, woven into the first two units' slots as the
    t-major-prepacked xv chunks land. The output projection does the same
    at the tail (po tiles rotate through the scores pool), so out tiles
    for the first q-half overlap the last unit's PV.
  * softmax denominators ride row hd of each otp via a ones-column
    appended to the PV stationary (v_sb tiles are [128, hd+1], ones
    preset once); per unit: drow -> reciprocal_approx_fast (partition 0;
    the custom DVE op misreads base_partition != 0) -> gpsimd
    partition_broadcast -> DVE multiply into OT.
  * a configurable number of exp tiles per unit run as Schraudolph int16
    exp on the DVE (i16 = round(s*1024*log2e + 15*1024 - 44) bitcast to
    fp16 ~= exp(s) within +-3%) to keep ACT's per-unit exp time under the
    PE's per-unit matmul time; the denominator stays consistent because
    the ones-column sums the same approximated values.
"""
from contextlib import ExitStack

import numpy as np

# Problem constants (hardcoded per harness contract).
B, S, D, H = 2, 2048, 1024, 16
HD = D // H          # 64
N_CORES = 8
GROUPS = N_CORES // B    # 4
H_LOC = H // GROUPS      # 4 heads per core
JJ = H_LOC * HD          # 256
P = 128

MM_DT = "fp16"
SCH_PER_UNIT = 3     # exp tiles per unit (units 2+) offloaded to DVE Schraudolph


def build_mha(s=S, d=D, h_loc=H_LOC, hd=HD, mm_dt=MM_DT, sch_per_unit=SCH_PER_UNIT,
              dbg=False):
    """Build + compile the per-core Bass program."""
    import concourse.bacc as bacc
    import concourse.tile as tile
    from concourse import mybir

    f32 = mybir.dt.float32
    i16 = mybir.dt.int16
    _two_byte = {"bf16": mybir.dt.bfloat16, "fp16": mybir.dt.float16}
    assert mm_dt in _two_byte
    mdt = _two_byte[mm_dt]
    Exp = mybir.ActivationFunctionType.Exp
    IdF = mybir.ActivationFunctionType.Identity
    MULT = mybir.AluOpType.mult
    ADD = mybir.AluOpType.add

    jj = h_loc * hd
    hd1 = hd + 1
    ktd = d // P             # contraction tiles per projection
    njt = (jj + P - 1) // P  # 128-row groups of the local head dims
    st_n = s // P            # key/seq tiles
    EH = 2                   # q-halves
    ec = s // EH             # q columns per unit
    nf = min(512, ec)        # psum-bank-sized matmul chunk
    efc = ec // nf           # matmul chunks per unit row
    pnf = min(512, s)        # projection chunk (>= 1 bank to keep acc groups apart)
    qnf = min(pnf, ec)       # q-half projection chunk
    ndo = (d + nf - 1) // nf
    # scores-pool tile width (also hosts V-proj + out-proj); bank-multiple
    spw = max(ec, d, jj, 512)
    otw = max(ec, 512)       # otp tile width, bank-multiple
    ppw = max(s, 512)        # projection psum width
    hpj = P // hd            # heads per j-group (2)

    nc = bacc.Bacc("TRN2", target_bir_lowering=False, debug=False)

    xq = nc.dram_tensor("xq", [P, ktd * s], mdt, kind="ExternalInput").ap()
    xk = nc.dram_tensor("xk", [P, ktd * s], mdt, kind="ExternalInput").ap()
    # xv is t-major: [p, (t, k, c)] so V-proj tile t is an early contiguous chunk
    xv = nc.dram_tensor("xv", [P, st_n * ktd * P], mdt, kind="ExternalInput").ap()
    wq = nc.dram_tensor("wq", [P, ktd * jj], mdt, kind="ExternalInput").ap()
    wk = nc.dram_tensor("wk", [P, ktd * jj], mdt, kind="ExternalInput").ap()
    wv = nc.dram_tensor("wv", [P, ktd * jj], mdt, kind="ExternalInput").ap()
    wo = nc.dram_tensor("wo", [P, njt * d], mdt, kind="ExternalInput").ap()
    bqp = nc.dram_tensor("bqp", [jj, 1], f32, kind="ExternalInput").ap()
    bkp = nc.dram_tensor("bkp", [jj, 1], f32, kind="ExternalInput").ap()
    out = nc.dram_tensor("out", [s, d], mdt, kind="ExternalOutput").ap()

    sch_a = float(1024 * np.log2(np.e))
    sch_b = float(15 * 1024 - 44)
    # Schraudolph tiles live in units >= 2 only (units 0-1 are PE-heavy with
    # the V/Q-e1 projection weaves, so ACT has slack there), spread evenly
    spu = max(0, min(sch_per_unit, st_n))
    sch_t = {int(st_n * (i + 0.5) / spu) for i in range(spu)} if spu else set()

    with tile.TileContext(nc) as tc, ExitStack() as ctx:
        persist = ctx.enter_context(tc.tile_pool(name="persist", bufs=1))

        qt_sb = [persist.tile([P, s], mdt, name=f"qt{j}", tag=f"qt{j}") for j in range(njt)]
        kt_sb = [persist.tile([P, s], mdt, name=f"kt{h}", tag=f"kt{h}") for h in range(h_loc)]
        ot_sb = [persist.tile([P, s], mdt, name=f"ot{j}", tag=f"ot{j}") for j in range(njt)]
        # PV stationaries [V_h | ones]; ones preset once in the preamble
        v_sb = [[persist.tile([P, hd1], mdt, name=f"v{t}_{h}", tag=f"v{t}_{h}")
                 for h in range(h_loc)] for t in range(st_n)]
        wq_b = persist.tile([P, ktd * jj], mdt, name="wq_b", tag="wq_b")
        wk_b = persist.tile([P, ktd * jj], mdt, name="wk_b", tag="wk_b")
        wv_b = persist.tile([P, ktd * jj], mdt, name="wv_b", tag="wv_b")
        wo_b = persist.tile([P, njt * d], mdt, name="wo_b", tag="wo_b")
        wq_r = [wq_b[:, k * jj:(k + 1) * jj] for k in range(ktd)]
        wk_r = [wk_b[:, k * jj:(k + 1) * jj] for k in range(ktd)]
        wv_r = [wv_b[:, k * jj:(k + 1) * jj] for k in range(ktd)]
        wo_r = [wo_b[:, j * d:(j + 1) * d] for j in range(njt)]
        bq_sb = persist.tile([P, njt], f32, name="bq_sb", tag="bq_sb")
        bk_sb = persist.tile([P, njt], f32, name="bk_sb", tag="bk_sb")
        wm_a = persist.tile([P, nf], mdt, name="wm_a", tag="wm_a")
        ep_t = persist.tile([1, 8], f32, name="ep_t", tag="ep_t")
        pbw = persist.tile([2, 8], f32, name="pbw", tag="pbw")

        # ---- preamble ----
        nc.vector.memset(ep_t[:], 0.0)
        nc.vector.memset(wm_a[:], 0.0)
        for j in range(njt):
            nc.scalar.dma_start(bq_sb[:, j:j + 1], bqp[j * P:(j + 1) * P, :])
            nc.scalar.dma_start(bk_sb[:, j:j + 1], bkp[j * P:(j + 1) * P, :])

        # input DMAs in consumption order across two queues (sync + gpsimd);
        # the scalar queue stays clear (two tiny bias DMAs) so ACT runs exp.
        nc.sync.dma_start(wk_b[:], wk)
        nc.gpsimd.dma_start(wq_b[:], wq)

        # PE warmup burst (runs while the first x tiles stream in)
        with tc.tile_pool(name="wup", bufs=1, space="PSUM") as wup:
            wm_p = wup.tile([P, nf], f32, name="wm_p", tag="wm_p")
            for i in range(8):
                nc.tensor.matmul(wm_p[:], wm_a[:, 0:P], wm_a[:], start=True, stop=True)
            nc.vector.tensor_copy(ep_t[0:1, 0:8], wm_p[0:1, 0:8])

        with tc.tile_pool(name="xrpool", bufs=2) as xrpool:
            xkb = xrpool.tile([P, ktd * s], mdt, name="xkb", tag="xbig")
            xqb = xrpool.tile([P, ktd * s], mdt, name="xqb", tag="xbig")
            # k-slice interleave across both queues: each tensor streams at
            # the full aggregate DMA rate, in consumption order
            for k in range(ktd):
                cs = slice(k * s, (k + 1) * s)
                eng = nc.sync if k % 2 == 0 else nc.gpsimd
                eng.dma_start(xkb[:, cs], xk[:, cs])
            # xq halves: e0 columns first so the e0 Q projection (and with
            # it the first scores/exp) starts before the e1 half lands
            for e in range(EH):
                for k in range(ktd):
                    cs = slice(k * s + e * ec, k * s + (e + 1) * ec)
                    eng = nc.gpsimd if k % 2 == 0 else nc.sync
                    eng.dma_start(xqb[:, cs], xq[:, cs])
            nc.gpsimd.dma_start(wv_b[:], wv)
            xk_t = [xkb[:, k * s:(k + 1) * s] for k in range(ktd)]
            xq_t = [xqb[:, k * s:(k + 1) * s] for k in range(ktd)]

            # exp-table preload after the early DMA triggers
            nc.scalar.activation(ep_t[:], ep_t[:], Exp)

            # ---- K projection + e0-half Q projection ----
            # One shared pool, bufs=2: consecutive [P, s] psum tiles land in
            # alternating bank sets, so each evacuation overlaps the next
            # projection's matmuls instead of serializing on a bank WAR.
            sc = float(1.0 / np.sqrt(hd))
            with tc.tile_pool(name="prpool", bufs=2, space="PSUM") as prpool:
                # K: k-outer, j-inner -- each arriving xk slice is consumed
                # fully (both j groups) so the PE tracks the DMA stream; both
                # j psums live simultaneously in the pool's two bank sets.
                ppk = [prpool.tile([P, ppw], f32, name=f"ppk{j}", tag="ppj")
                       for j in range(njt)]
                for k in range(ktd):
                    for j in range(njt):
                        for c in range(s // pnf):
                            nc.tensor.matmul(
                                ppk[j][:, c * pnf:(c + 1) * pnf],
                                wk_r[k][:, j * P:(j + 1) * P],
                                xk_t[k][:, c * pnf:(c + 1) * pnf],
                                start=(k == 0), stop=(k == ktd - 1))
                    if k == 0:
                        # gpsimd prep after the first K matmuls are emitted:
                        # zero rows for the padded per-head KT tiles, the
                        # partition_broadcast ext-isa lib warmup (~6us IRAM),
                        # and the PV-stationary ones columns.
                        for h in range(h_loc):
                            off = (h * hd) % P
                            if off > 0:
                                nc.gpsimd.memset(kt_sb[h][0:off, :], 0.0)
                            if off + hd < P:
                                nc.gpsimd.memset(kt_sb[h][off + hd:P, :], 0.0)
                        nc.gpsimd.partition_broadcast(pbw[:], ep_t[0:1, :], channels=2)
                        for t in range(st_n):
                            for h in range(h_loc):
                                nc.gpsimd.memset(v_sb[t][h][:, hd:hd1], 1.0)
                # kt evacuations split DVE/ACT so both j psums free fast
                for j in range(njt):
                    for hh in range(hpj):
                        h = j * hpj + hh
                        if h >= h_loc:
                            continue
                        r0 = hh * hd
                        if hh % 2 == 0:
                            nc.vector.tensor_scalar(
                                kt_sb[h][r0:r0 + hd, :], ppk[j][r0:r0 + hd, 0:s],
                                1.0, bk_sb[r0:r0 + hd, j:j + 1], op0=MULT, op1=ADD)
                        else:
                            nc.scalar.activation(
                                kt_sb[h][r0:r0 + hd, :], ppk[j][r0:r0 + hd, 0:s],
                                IdF, bias=bk_sb[r0:r0 + hd, j:j + 1], scale=1.0)

                # e0-half Q projection (the e1 half runs through the scores
                # pool, woven into unit 0 once its xq columns land)
                for j in range(njt):
                    ppq = prpool.tile([P, ppw], f32, name=f"ppq0{j}", tag="ppj")
                    for k in range(ktd):
                        for c in range(ec // qnf):
                            nc.tensor.matmul(
                                ppq[:, c * qnf:(c + 1) * qnf],
                                wq_r[k][:, j * P:(j + 1) * P],
                                xq_t[k][:, c * qnf:(c + 1) * qnf],
                                start=(k == 0), stop=(k == ktd - 1))
                    if j % 2 == 0:
                        nc.vector.tensor_scalar(
                            qt_sb[j][:, 0:ec], ppq[:, 0:ec], sc, bq_sb[:, j:j + 1],
                            op0=MULT, op1=ADD)
                    else:
                        nc.scalar.activation(
                            qt_sb[j][:, 0:ec], ppq[:, 0:ec], IdF,
                            bias=bq_sb[:, j:j + 1], scale=sc)

            # xv lands in xk's slot (WAR: waits for the K projection);
            # t-major chunks split across both queues so early V tiles land
            # as the first units run.
            xvb = xrpool.tile([P, st_n * ktd * P], mdt, name="xvb", tag="xbig")
            nxv = min(4, st_n)
            tpc = st_n // nxv
            cw = tpc * ktd * P
            for ci in range(nxv):
                eng = nc.gpsimd if ci % 2 == 0 else nc.sync
                eng.dma_start(xvb[:, ci * cw:(ci + 1) * cw],
                              xv[:, ci * cw:(ci + 1) * cw])
            nc.sync.dma_start(wo_b[:], wo)

            def xv_st(t, k):  # V-proj stationary chunk
                return xvb[:, (t * ktd + k) * P:(t * ktd + k + 1) * P]

            # ---- attention: units (e outer, h inner), flat pipeline ----
            units = [(e, h) for e in range(EH) for h in range(h_loc)]
            nu = len(units)
            with tc.tile_pool(name="sppool", bufs=2, space="PSUM") as sppool, \
                 tc.tile_pool(name="opsum", bufs=2, space="PSUM") as opsum, \
                 tc.tile_pool(name="ptpool", bufs=min(12, 2 * st_n + 4)) as ptpool, \
                 tc.tile_pool(name="npool", bufs=2) as npool, \
                 tc.tile_pool(name="fout", bufs=3) as fout:
                pts = {}
                otps = {}
                obs = {}

                def scores(u, t):
                    e, h = units[u]
                    sp = sppool.tile([P, spw], f32, name=f"sp{u}_{t}", tag="sp")
                    for f in range(efc):
                        q0 = e * ec + f * nf
                        nc.tensor.matmul(
                            sp[:, f * nf:(f + 1) * nf],
                            kt_sb[h][:, t * P:(t + 1) * P],
                            qt_sb[(h * hd) // P][:, q0:q0 + nf],
                            start=True, stop=True)
                    pt = ptpool.tile([P, ec], mdt, name=f"pt{u}_{t}", tag="pt")
                    if t in sch_t and u >= 2:
                        nc.vector.tensor_scalar(pt.bitcast(i16)[:], sp[:, 0:ec],
                                                sch_a, sch_b, op0=MULT, op1=ADD)
                    else:
                        nc.scalar.activation(pt[:], sp[:, 0:ec], Exp)
                    pts[u, t] = pt

                def qproj_e1(j):
                    ppq = sppool.tile([P, spw], f32, name=f"ppq1{j}", tag="sp")
                    for k in range(ktd):
                        for c in range(ec // qnf):
                            nc.tensor.matmul(
                                ppq[:, c * qnf:(c + 1) * qnf],
                                wq_r[k][:, j * P:(j + 1) * P],
                                xq_t[k][:, ec + c * qnf:ec + (c + 1) * qnf],
                                start=(k == 0), stop=(k == ktd - 1))
                    nc.vector.tensor_scalar(
                        qt_sb[j][:, ec:2 * ec], ppq[:, 0:ec], sc,
                        bq_sb[:, j:j + 1], op0=MULT, op1=ADD)

                def vproj(t):
                    vp = sppool.tile([P, spw], f32, name=f"vp{t}", tag="sp")
                    for k in range(ktd):
                        nc.tensor.matmul(vp[:, 0:jj], xv_st(t, k), wv_r[k][:],
                                         start=(k == 0), stop=(k == ktd - 1))
                    for h in range(h_loc):
                        nc.vector.tensor_copy(v_sb[t][h][:, 0:hd],
                                              vp[:, h * hd:(h + 1) * hd])

                def pv(u, t):
                    if t == 0:
                        otps[u] = opsum.tile([P, otw], f32, name=f"otp{u}", tag="otp")
                    otp = otps[u]
                    e, h = units[u]
                    pt = pts.pop((u, t))
                    for f in range(efc):
                        nc.tensor.matmul(
                            otp[0:hd1, f * nf:(f + 1) * nf],
                            v_sb[t][h][:],
                            pt[:, f * nf:(f + 1) * nf],
                            start=(t == 0), stop=(t == st_n - 1))

                def evac(u):
                    # numerators + denominator row out of PSUM, recip, bcast
                    last = u == nu - 1
                    otp = otps.pop(u)
                    drow = npool.tile([1, ec], f32, name=f"drow{u}", tag="drow", bufs=1)
                    if last:
                        nc.scalar.copy(drow[:], otp[hd:hd1, 0:ec])
                    else:
                        nc.vector.tensor_copy(drow[:], otp[hd:hd1, 0:ec])
                    ob = npool.tile([hd, ec], f32, name=f"ob{u}", tag="ob")
                    nc.vector.tensor_copy(ob[:], otp[0:hd, 0:ec])
                    rrow = npool.tile([1, ec], f32, name=f"rrow{u}", tag="rrow", bufs=1)
                    nc.vector.reciprocal_approx_fast(rrow[:], drow[:])
                    bb = npool.tile([hd, ec], f32, name=f"bb{u}", tag="bb")
                    nc.gpsimd.partition_broadcast(bb[:], rrow[:], channels=hd)
                    obs[u] = (ob, bb)

                def norm(u):
                    e, h = units[u]
                    ob, bb = obs.pop(u)
                    jt, off = (h * hd) // P, (h * hd) % P
                    nc.vector.tensor_mul(
                        ot_sb[jt][off:off + hd, e * ec:(e + 1) * ec],
                        ob[:], bb[:])

                def outproj(t):
                    po = sppool.tile([P, spw], f32, name=f"po{t}", tag="sp")
                    for c in range(ndo):
                        for j in range(njt):
                            nc.tensor.matmul(
                                po[:, c * nf:(c + 1) * nf],
                                ot_sb[j][:, t * P:(t + 1) * P],
                                wo_r[j][:, c * nf:(c + 1) * nf],
                                start=(j == 0), stop=(j == njt - 1))
                    ob = fout.tile([P, d], mdt, name=f"fo{t}", tag="fo")
                    # woven e0 copies go to DVE (ACT still exping); drain
                    # copies alternate engines so neither gates the sp pool
                    if t < st_n // 2 and st_n >= 8:
                        nc.vector.tensor_copy(ob[:], po[:, 0:d])
                    elif t % 2 == 0:
                        nc.scalar.copy(ob[:], po[:, 0:d])
                    else:
                        nc.vector.tensor_copy(ob[:], po[:, 0:d])
                    oeng = nc.sync if t % 2 == 0 else nc.gpsimd
                    oeng.dma_start(out[t * P:(t + 1) * P, :], ob[:])

                # flat pipeline: the PV stream runs LAG slots behind the
                # scores stream (crossing unit boundaries), so the drain
                # after the last scores is only LAG slots + the final
                # normalize chain. V-proj tiles weave 1:1 into slots 2..,
                # paced to the xv chunk arrivals; the e1 Q projection takes
                # unit 0's first slots; the e0 half of the out-projection
                # weaves into unit nu-2 once all e0 norms are emitted.
                LAG = min(4, st_n)
                evt = min(LAG, st_n - 1)
                nrt = min(8, st_n - 1)
                oq = list(range(st_n))
                VOFF = njt  # vproj weave starts after the qproj_e1 slots
                for u in range(nu):
                    for t in range(st_n):
                        m = u * st_n + t
                        scores(u, t)
                        if u == 0 and t < njt:
                            qproj_e1(t)
                        if VOFF <= m < VOFF + st_n:
                            vproj(m - VOFF)
                        if u >= 1 and t < LAG:
                            pv(u - 1, st_n - LAG + t)
                        if t >= LAG:
                            pv(u, t - LAG)
                        if u >= 1 and t == evt:
                            evac(u - 1)
                        if u >= 1 and t == nrt:
                            norm(u - 1)
                        if u == nu - 2 and st_n >= 8 and t % 2 == 0:
                            outproj(oq.pop(0))  # e0 tiles; norms long done
                # drain: last LAG PV slots + final chain + e1 out-proj
                for t in range(st_n - LAG, st_n):
                    pv(nu - 1, t)
                evac(nu - 1)
                norm(nu - 1)
                while oq:
                    outproj(oq.pop(0))

    nc.compile()
    return nc


_NC_CACHE = {}


def _get_nc():
    key = (MM_DT, SCH_PER_UNIT)
    if key not in _NC_CACHE:
        _NC_CACHE[key] = build_mha(mm_dt=MM_DT, sch_per_unit=SCH_PER_UNIT)
    return _NC_CACHE[key]


def pack_x(xT, ktd=None, p=P):
    """[d, s] -> [p, (k s)] contiguous DMA layout."""
    d, s = xT.shape
    k = d // p
    return np.ascontiguousarray(xT.reshape(k, p, s).transpose(1, 0, 2).reshape(p, k * s))


def pack_xv(xT, st_n=None, p=P):
    """[d, s] -> [p, (t k c)] t-major layout for early V-proj tiles."""
    d, s = xT.shape
    k = d // p
    t = s // p
    return np.ascontiguousarray(
        xT.reshape(k, p, t, p).transpose(1, 2, 0, 3).reshape(p, t * k * p))


def pack_w(wT, p=P):
    """[d, jj] -> [p, (k jj)]."""
    d, jj = wT.shape
    k = d // p
    return np.ascontiguousarray(wT.reshape(k, p, jj).transpose(1, 0, 2).reshape(p, k * jj))


def build_in_maps(inputs, mm_dt=MM_DT):
    if mm_dt == "bf16":
        import ml_dtypes
        xdt = ml_dtypes.bfloat16
    else:
        xdt = np.float16

    q = np.asarray(inputs["query"], np.float32)
    k = np.asarray(inputs.get("key_", inputs.get("key")), np.float32)
    v = np.asarray(inputs["value"], np.float32)
    Wq = np.asarray(inputs["Wq"], np.float32)
    Wk = np.asarray(inputs["Wk"], np.float32)
    Wv = np.asarray(inputs["Wv"], np.float32)
    Wo = np.asarray(inputs["Wo"], np.float32)
    bq = np.asarray(inputs["bq"], np.float32)
    bk = np.asarray(inputs["bk"], np.float32)

    sc = np.float32(1.0 / np.sqrt(HD))

    qP = [pack_x(q[b].T.astype(xdt)) for b in range(B)]
    kP = [pack_x(k[b].T.astype(xdt)) for b in range(B)]
    vP = [pack_xv(v[b].T.astype(xdt)) for b in range(B)]
    WqT, WkT, WvT = Wq.T, Wk.T, Wv.T

    in_maps = []
    for core in range(N_CORES):
        b, g = divmod(core, GROUPS)
        sl = slice(g * JJ, (g + 1) * JJ)
        in_maps.append({
            "xq": qP[b],
            "xk": kP[b],
            "xv": vP[b],
            "wq": pack_w(np.ascontiguousarray(WqT[:, sl]).astype(xdt)),
            "wk": pack_w(np.ascontiguousarray(WkT[:, sl]).astype(xdt)),
            "wv": pack_w(np.ascontiguousarray(WvT[:, sl]).astype(xdt)),
            "wo": pack_w(np.ascontiguousarray(Wo[:, sl].T).astype(xdt)),
            "bqp": np.ascontiguousarray((bq[sl] * sc)[:, None]),
            "bkp": np.ascontiguousarray(bk[sl][:, None]),
        })
    return in_maps


def combine_outputs(results, inputs):
    Wo = np.asarray(inputs["Wo"], np.float32)
    bv = np.asarray(inputs["bv"], np.float32)
    bo = np.asarray(inputs["bo"], np.float32)
    const = bv @ Wo.T + bo  # exact host-side bias correction
    outp = np.empty((B, S, D), np.float32)
    for b in range(B):
        acc = results[b * GROUPS]["out"].astype(np.float32)
        for g in range(1, GROUPS):
            acc = acc + results[b * GROUPS + g]["out"].astype(np.float32)
        outp[b] = acc + const[None, :]
    return outp


def kernel(**inputs):
    import time
    from concourse.bass_utils import run_bass_kernel_spmd

    nc = _get_nc()
    in_maps = build_in_maps(inputs)
    last_err = None
    for attempt in range(3):
        try:
            res = run_bass_kernel_spmd(nc, in_maps, list(range(N_CORES)))
            return combine_outputs(res.results, inputs)
        except Exception as e:  # transient device wedge: retry
            last_err = e
            try:
                import jax
                import jax.numpy as jnp
                for dvc in jax.devices()[:N_CORES]:
                    jax.device_put(jnp.zeros((8, 8)), dvc).block_until_ready()
            except Exception:
                pass
            time.sleep(5.0 * (attempt + 1))
    raise last_err


# revision 14
# speedup vs baseline: 1.0361x; 1.0361x over previous
"""Multi-head attention (B=2, S=2048, D=1024, H=16) on 8 TRN2 NeuronCores.

Sharding: core = (batch b, head-group g): 2 batches x 4 groups of 4 heads.
Each core computes its group's QKV projections, attention, and a partial
output projection; the host sums the 4 partials per batch and adds the
exact bias constant (bv @ Wo.T + bo). bq/bk are applied on device.

v2 schedule (derived from the v1 trace: ACT exp 150us was the bottleneck
and only started at 75us because xq/wq landed last on the DMA queues; the
tail serialized 44us of evac->recip->broadcast->norm->out-proj):

  * all host-side tensors are prepacked into [128, N] row-major layouts so
    every DMA is a plain contiguous 2D copy (the v1 512B-elem rearranged
    weight DMAs ran at ~56GB/s and stalled the first projection).
  * two HWDGE queues (sync + gpsimd) stream inputs in consumption order:
    K then Q inputs first (9MB -> exp starts ~30us), xv/wv behind them,
    wo last. The scalar queue carries no DMA so ACT only runs exp.
  * K and Q projections are j-split (4 PSUM banks each) so the kt/qt
    evacuations pipeline with the next projection's matmuls.
  * attention runs in 8 units (q-half e outer, head h inner), each unit =
    16 key-tiles: scores -> exp[128,1024] -> PV into a double-buffered
    half-otp [128,1024]. The scores stream runs one unit ahead of the PV
    stream. PSUM: sp 2x2 banks + otp 2x2 banks = 8.
  * the V projection has no PSUM of its own: its [128,jj] tiles rotate
    through the scores pool, woven into the first two units' slots as the
    t-major-prepacked xv chunks land. The output projection does the same
    at the tail (po tiles rotate through the scores pool), so out tiles
    for the first q-half overlap the last unit's PV.
  * softmax denominators ride row hd of each otp via a ones-column
    appended to the PV stationary (v_sb tiles are [128, hd+1], ones
    preset once); per unit: drow -> reciprocal_approx_fast (partition 0;
    the custom DVE op misreads base_partition != 0) -> gpsimd
    partition_broadcast -> DVE multiply into OT.
  * a configurable number of exp tiles per unit run as Schraudolph int16
    exp on the DVE (i16 = round(s*1024*log2e + 15*1024 - 44) bitcast to
    fp16 ~= exp(s) within +-3%) to keep ACT's per-unit exp time under the
    PE's per-unit matmul time; the denominator stays consistent because
    the ones-column sums the same approximated values.
"""
from contextlib import ExitStack

import numpy as np

# Problem constants (hardcoded per harness contract).
B, S, D, H = 2, 2048, 1024, 16
HD = D // H          # 64
N_CORES = 8
GROUPS = N_CORES // B    # 4
H_LOC = H // GROUPS      # 4 heads per core
JJ = H_LOC * HD          # 256
P = 128

MM_DT = "fp16"
SCH_PER_UNIT = 4     # exp tiles per unit (units 2+) offloaded to DVE Schraudolph


def build_mha(s=S, d=D, h_loc=H_LOC, hd=HD, mm_dt=MM_DT, sch_per_unit=SCH_PER_UNIT,
              dbg=False):
    """Build + compile the per-core Bass program."""
    import concourse.bacc as bacc
    import concourse.tile as tile
    from concourse import mybir

    f32 = mybir.dt.float32
    i16 = mybir.dt.int16
    _two_byte = {"bf16": mybir.dt.bfloat16, "fp16": mybir.dt.float16}
    assert mm_dt in _two_byte
    mdt = _two_byte[mm_dt]
    Exp = mybir.ActivationFunctionType.Exp
    IdF = mybir.ActivationFunctionType.Identity
    MULT = mybir.AluOpType.mult
    ADD = mybir.AluOpType.add

    jj = h_loc * hd
    hd1 = hd + 1
    ktd = d // P             # contraction tiles per projection
    njt = (jj + P - 1) // P  # 128-row groups of the local head dims
    st_n = s // P            # key/seq tiles
    EH = 2                   # q-halves
    ec = s // EH             # q columns per unit
    nf = min(512, ec)        # psum-bank-sized matmul chunk
    efc = ec // nf           # matmul chunks per unit row
    pnf = min(512, s)        # projection chunk (>= 1 bank to keep acc groups apart)
    qnf = min(pnf, ec)       # q-half projection chunk
    ndo = (d + nf - 1) // nf
    # scores-pool tile width (also hosts V-proj + out-proj); bank-multiple
    spw = max(ec, d, jj, 512)
    otw = max(ec, 512)       # otp tile width, bank-multiple
    ppw = max(s, 512)        # projection psum width
    hpj = P // hd            # heads per j-group (2)

    nc = bacc.Bacc("TRN2", target_bir_lowering=False, debug=False)

    xq = nc.dram_tensor("xq", [P, ktd * s], mdt, kind="ExternalInput").ap()
    xk = nc.dram_tensor("xk", [P, ktd * s], mdt, kind="ExternalInput").ap()
    # xv is t-major: [p, (t, k, c)] so V-proj tile t is an early contiguous chunk
    xv = nc.dram_tensor("xv", [P, st_n * ktd * P], mdt, kind="ExternalInput").ap()
    wq = nc.dram_tensor("wq", [P, ktd * jj], mdt, kind="ExternalInput").ap()
    wk = nc.dram_tensor("wk", [P, ktd * jj], mdt, kind="ExternalInput").ap()
    wv = nc.dram_tensor("wv", [P, ktd * jj], mdt, kind="ExternalInput").ap()
    wo = nc.dram_tensor("wo", [P, njt * d], mdt, kind="ExternalInput").ap()
    bqp = nc.dram_tensor("bqp", [jj, 1], f32, kind="ExternalInput").ap()
    bkp = nc.dram_tensor("bkp", [jj, 1], f32, kind="ExternalInput").ap()
    out = nc.dram_tensor("out", [s, d], mdt, kind="ExternalOutput").ap()

    sch_a = float(1024 * np.log2(np.e))
    sch_b = float(15 * 1024 - 44)
    # Schraudolph tiles live in units >= 2 only (units 0-1 are PE-heavy with
    # the V/Q-e1 projection weaves, so ACT has slack there), spread evenly
    spu = max(0, min(sch_per_unit, st_n))
    sch_t = {int(st_n * (i + 0.5) / spu) for i in range(spu)} if spu else set()

    with tile.TileContext(nc) as tc, ExitStack() as ctx:
        persist = ctx.enter_context(tc.tile_pool(name="persist", bufs=1))

        qt_sb = [persist.tile([P, s], mdt, name=f"qt{j}", tag=f"qt{j}") for j in range(njt)]
        kt_sb = [persist.tile([P, s], mdt, name=f"kt{h}", tag=f"kt{h}") for h in range(h_loc)]
        ot_sb = [persist.tile([P, s], mdt, name=f"ot{j}", tag=f"ot{j}") for j in range(njt)]
        # PV stationaries [V_h | ones]; ones preset once in the preamble
        v_sb = [[persist.tile([P, hd1], mdt, name=f"v{t}_{h}", tag=f"v{t}_{h}")
                 for h in range(h_loc)] for t in range(st_n)]
        wq_b = persist.tile([P, ktd * jj], mdt, name="wq_b", tag="wq_b")
        wk_b = persist.tile([P, ktd * jj], mdt, name="wk_b", tag="wk_b")
        wv_b = persist.tile([P, ktd * jj], mdt, name="wv_b", tag="wv_b")
        wo_b = persist.tile([P, njt * d], mdt, name="wo_b", tag="wo_b")
        wq_r = [wq_b[:, k * jj:(k + 1) * jj] for k in range(ktd)]
        wk_r = [wk_b[:, k * jj:(k + 1) * jj] for k in range(ktd)]
        wv_r = [wv_b[:, k * jj:(k + 1) * jj] for k in range(ktd)]
        wo_r = [wo_b[:, j * d:(j + 1) * d] for j in range(njt)]
        bq_sb = persist.tile([P, njt], f32, name="bq_sb", tag="bq_sb")
        bk_sb = persist.tile([P, njt], f32, name="bk_sb", tag="bk_sb")
        wm_a = persist.tile([P, nf], mdt, name="wm_a", tag="wm_a")
        ep_t = persist.tile([1, 8], f32, name="ep_t", tag="ep_t")
        pbw = persist.tile([2, 8], f32, name="pbw", tag="pbw")

        # ---- preamble ----
        nc.vector.memset(ep_t[:], 0.0)
        nc.vector.memset(wm_a[:], 0.0)
        for j in range(njt):
            nc.scalar.dma_start(bq_sb[:, j:j + 1], bqp[j * P:(j + 1) * P, :])
            nc.scalar.dma_start(bk_sb[:, j:j + 1], bkp[j * P:(j + 1) * P, :])

        # input DMAs in consumption order across two queues (sync + gpsimd);
        # the scalar queue stays clear (two tiny bias DMAs) so ACT runs exp.
        nc.sync.dma_start(wk_b[:], wk)
        nc.gpsimd.dma_start(wq_b[:], wq)

        # PE warmup burst (runs while the first x tiles stream in)
        with tc.tile_pool(name="wup", bufs=1, space="PSUM") as wup:
            wm_p = wup.tile([P, nf], f32, name="wm_p", tag="wm_p")
            for i in range(16):
                nc.tensor.matmul(wm_p[:], wm_a[:, 0:P], wm_a[:], start=True, stop=True)
            nc.vector.tensor_copy(ep_t[0:1, 0:8], wm_p[0:1, 0:8])

        with tc.tile_pool(name="xrpool", bufs=2) as xrpool:
            xkb = xrpool.tile([P, ktd * s], mdt, name="xkb", tag="xbig")
            xqb = xrpool.tile([P, ktd * s], mdt, name="xqb", tag="xbig")
            # k-slice interleave across both queues: each tensor streams at
            # the full aggregate DMA rate, in consumption order
            for k in range(ktd):
                cs = slice(k * s, (k + 1) * s)
                eng = nc.sync if k % 2 == 0 else nc.gpsimd
                eng.dma_start(xkb[:, cs], xk[:, cs])
            # xq halves: e0 columns first so the e0 Q projection (and with
            # it the first scores/exp) starts before the e1 half lands
            for e in range(EH):
                for k in range(ktd):
                    cs = slice(k * s + e * ec, k * s + (e + 1) * ec)
                    eng = nc.gpsimd if k % 2 == 0 else nc.sync
                    eng.dma_start(xqb[:, cs], xq[:, cs])
            nc.gpsimd.dma_start(wv_b[:], wv)
            xk_t = [xkb[:, k * s:(k + 1) * s] for k in range(ktd)]
            xq_t = [xqb[:, k * s:(k + 1) * s] for k in range(ktd)]

            # exp-table preload after the early DMA triggers
            nc.scalar.activation(ep_t[:], ep_t[:], Exp)

            # ---- K projection + e0-half Q projection ----
            # One shared pool, bufs=2: consecutive [P, s] psum tiles land in
            # alternating bank sets, so each evacuation overlaps the next
            # projection's matmuls instead of serializing on a bank WAR.
            sc = float(1.0 / np.sqrt(hd))
            with tc.tile_pool(name="prpool", bufs=2, space="PSUM") as prpool:
                # K: k-outer, j-inner -- each arriving xk slice is consumed
                # fully (both j groups) so the PE tracks the DMA stream; both
                # j psums live simultaneously in the pool's two bank sets.
                ppk = [prpool.tile([P, ppw], f32, name=f"ppk{j}", tag="ppj")
                       for j in range(njt)]
                for k in range(ktd):
                    for j in range(njt):
                        for c in range(s // pnf):
                            nc.tensor.matmul(
                                ppk[j][:, c * pnf:(c + 1) * pnf],
                                wk_r[k][:, j * P:(j + 1) * P],
                                xk_t[k][:, c * pnf:(c + 1) * pnf],
                                start=(k == 0), stop=(k == ktd - 1))
                    if k == 0:
                        # gpsimd prep after the first K matmuls are emitted:
                        # zero rows for the padded per-head KT tiles, the
                        # partition_broadcast ext-isa lib warmup (~6us IRAM),
                        # and the PV-stationary ones columns.
                        for h in range(h_loc):
                            off = (h * hd) % P
                            if off > 0:
                                nc.gpsimd.memset(kt_sb[h][0:off, :], 0.0)
                            if off + hd < P:
                                nc.gpsimd.memset(kt_sb[h][off + hd:P, :], 0.0)
                        nc.gpsimd.partition_broadcast(pbw[:], ep_t[0:1, :], channels=2)
                        for t in range(st_n):
                            for h in range(h_loc):
                                nc.gpsimd.memset(v_sb[t][h][:, hd:hd1], 1.0)
                # kt evacuations split DVE/ACT so both j psums free fast
                for j in range(njt):
                    for hh in range(hpj):
                        h = j * hpj + hh
                        if h >= h_loc:
                            continue
                        r0 = hh * hd
                        if hh % 2 == 0:
                            nc.vector.tensor_scalar(
                                kt_sb[h][r0:r0 + hd, :], ppk[j][r0:r0 + hd, 0:s],
                                1.0, bk_sb[r0:r0 + hd, j:j + 1], op0=MULT, op1=ADD)
                        else:
                            nc.scalar.activation(
                                kt_sb[h][r0:r0 + hd, :], ppk[j][r0:r0 + hd, 0:s],
                                IdF, bias=bk_sb[r0:r0 + hd, j:j + 1], scale=1.0)

                # e0-half Q projection (the e1 half runs through the scores
                # pool, woven into unit 0 once its xq columns land)
                for j in range(njt):
                    ppq = prpool.tile([P, ppw], f32, name=f"ppq0{j}", tag="ppj")
                    for k in range(ktd):
                        for c in range(ec // qnf):
                            nc.tensor.matmul(
                                ppq[:, c * qnf:(c + 1) * qnf],
                                wq_r[k][:, j * P:(j + 1) * P],
                                xq_t[k][:, c * qnf:(c + 1) * qnf],
                                start=(k == 0), stop=(k == ktd - 1))
                    if j % 2 == 0:
                        nc.vector.tensor_scalar(
                            qt_sb[j][:, 0:ec], ppq[:, 0:ec], sc, bq_sb[:, j:j + 1],
                            op0=MULT, op1=ADD)
                    else:
                        nc.scalar.activation(
                            qt_sb[j][:, 0:ec], ppq[:, 0:ec], IdF,
                            bias=bq_sb[:, j:j + 1], scale=sc)

            # xv lands in xk's slot (WAR: waits for the K projection);
            # t-major chunks split across both queues so early V tiles land
            # as the first units run.
            xvb = xrpool.tile([P, st_n * ktd * P], mdt, name="xvb", tag="xbig")
            nxv = min(4, st_n)
            tpc = st_n // nxv
            cw = tpc * ktd * P
            for ci in range(nxv):
                eng = nc.gpsimd if ci % 2 == 0 else nc.sync
                eng.dma_start(xvb[:, ci * cw:(ci + 1) * cw],
                              xv[:, ci * cw:(ci + 1) * cw])
            nc.sync.dma_start(wo_b[:], wo)

            def xv_st(t, k):  # V-proj stationary chunk
                return xvb[:, (t * ktd + k) * P:(t * ktd + k + 1) * P]

            # ---- attention: units (e outer, h inner), flat pipeline ----
            units = [(e, h) for e in range(EH) for h in range(h_loc)]
            nu = len(units)
            with tc.tile_pool(name="sppool", bufs=2, space="PSUM") as sppool, \
                 tc.tile_pool(name="opsum", bufs=2, space="PSUM") as opsum, \
                 tc.tile_pool(name="ptpool", bufs=min(12, 2 * st_n + 4)) as ptpool, \
                 tc.tile_pool(name="npool", bufs=2) as npool, \
                 tc.tile_pool(name="fout", bufs=3) as fout:
                pts = {}
                otps = {}
                obs = {}

                def scores(u, t):
                    e, h = units[u]
                    sp = sppool.tile([P, spw], f32, name=f"sp{u}_{t}", tag="sp")
                    for f in range(efc):
                        q0 = e * ec + f * nf
                        nc.tensor.matmul(
                            sp[:, f * nf:(f + 1) * nf],
                            kt_sb[h][:, t * P:(t + 1) * P],
                            qt_sb[(h * hd) // P][:, q0:q0 + nf],
                            start=True, stop=True)
                    pt = ptpool.tile([P, ec], mdt, name=f"pt{u}_{t}", tag="pt")
                    if t in sch_t and u >= 2:
                        nc.vector.tensor_scalar(pt.bitcast(i16)[:], sp[:, 0:ec],
                                                sch_a, sch_b, op0=MULT, op1=ADD)
                    else:
                        nc.scalar.activation(pt[:], sp[:, 0:ec], Exp)
                    pts[u, t] = pt

                def qproj_e1(j):
                    ppq = sppool.tile([P, spw], f32, name=f"ppq1{j}", tag="sp")
                    for k in range(ktd):
                        for c in range(ec // qnf):
                            nc.tensor.matmul(
                                ppq[:, c * qnf:(c + 1) * qnf],
                                wq_r[k][:, j * P:(j + 1) * P],
                                xq_t[k][:, ec + c * qnf:ec + (c + 1) * qnf],
                                start=(k == 0), stop=(k == ktd - 1))
                    nc.vector.tensor_scalar(
                        qt_sb[j][:, ec:2 * ec], ppq[:, 0:ec], sc,
                        bq_sb[:, j:j + 1], op0=MULT, op1=ADD)

                def vproj(t):
                    vp = sppool.tile([P, spw], f32, name=f"vp{t}", tag="sp")
                    for k in range(ktd):
                        nc.tensor.matmul(vp[:, 0:jj], xv_st(t, k), wv_r[k][:],
                                         start=(k == 0), stop=(k == ktd - 1))
                    for h in range(h_loc):
                        nc.vector.tensor_copy(v_sb[t][h][:, 0:hd],
                                              vp[:, h * hd:(h + 1) * hd])

                def pv(u, t):
                    if t == 0:
                        otps[u] = opsum.tile([P, otw], f32, name=f"otp{u}", tag="otp")
                    otp = otps[u]
                    e, h = units[u]
                    pt = pts.pop((u, t))
                    for f in range(efc):
                        nc.tensor.matmul(
                            otp[0:hd1, f * nf:(f + 1) * nf],
                            v_sb[t][h][:],
                            pt[:, f * nf:(f + 1) * nf],
                            start=(t == 0), stop=(t == st_n - 1))

                def evac(u):
                    # numerators + denominator row out of PSUM, recip, bcast
                    last = u == nu - 1
                    otp = otps.pop(u)
                    drow = npool.tile([1, ec], f32, name=f"drow{u}", tag="drow", bufs=1)
                    if last:
                        nc.scalar.copy(drow[:], otp[hd:hd1, 0:ec])
                    else:
                        nc.vector.tensor_copy(drow[:], otp[hd:hd1, 0:ec])
                    ob = npool.tile([hd, ec], f32, name=f"ob{u}", tag="ob")
                    nc.vector.tensor_copy(ob[:], otp[0:hd, 0:ec])
                    rrow = npool.tile([1, ec], f32, name=f"rrow{u}", tag="rrow", bufs=1)
                    nc.vector.reciprocal_approx_fast(rrow[:], drow[:])
                    bb = npool.tile([hd, ec], f32, name=f"bb{u}", tag="bb")
                    nc.gpsimd.partition_broadcast(bb[:], rrow[:], channels=hd)
                    obs[u] = (ob, bb)

                def norm(u):
                    e, h = units[u]
                    ob, bb = obs.pop(u)
                    jt, off = (h * hd) // P, (h * hd) % P
                    nc.vector.tensor_mul(
                        ot_sb[jt][off:off + hd, e * ec:(e + 1) * ec],
                        ob[:], bb[:])

                def outproj(t):
                    po = sppool.tile([P, spw], f32, name=f"po{t}", tag="sp")
                    for c in range(ndo):
                        for j in range(njt):
                            nc.tensor.matmul(
                                po[:, c * nf:(c + 1) * nf],
                                ot_sb[j][:, t * P:(t + 1) * P],
                                wo_r[j][:, c * nf:(c + 1) * nf],
                                start=(j == 0), stop=(j == njt - 1))
                    ob = fout.tile([P, d], mdt, name=f"fo{t}", tag="fo")
                    # alternate engines (out-proj runs in the drain, ACT free)
                    if t % 2 == 0:
                        nc.scalar.copy(ob[:], po[:, 0:d])
                    else:
                        nc.vector.tensor_copy(ob[:], po[:, 0:d])
                    oeng = nc.sync if t % 2 == 0 else nc.gpsimd
                    oeng.dma_start(out[t * P:(t + 1) * P, :], ob[:])

                # flat pipeline: the PV stream runs LAG slots behind the
                # scores stream (crossing unit boundaries), so the drain
                # after the last scores is only LAG slots + the final
                # normalize chain. V-proj tiles weave 1:1 into slots 2..,
                # paced to the xv chunk arrivals; the e1 Q projection takes
                # unit 0's first slots; the e0 half of the out-projection
                # weaves into unit nu-2 once all e0 norms are emitted.
                LAG = min(4, st_n)
                evt = min(LAG, st_n - 1)
                nrt = min(8, st_n - 1)
                oq = list(range(st_n))
                VOFF = njt  # vproj weave starts after the qproj_e1 slots
                for u in range(nu):
                    for t in range(st_n):
                        m = u * st_n + t
                        scores(u, t)
                        if u == 0 and t < njt:
                            qproj_e1(t)
                        if VOFF <= m < VOFF + st_n:
                            vproj(m - VOFF)
                        if u >= 1 and t < LAG:
                            pv(u - 1, st_n - LAG + t)
                        if t >= LAG:
                            pv(u, t - LAG)
                        if u >= 1 and t == evt:
                            evac(u - 1)
                        if u >= 1 and t == nrt:
                            norm(u - 1)
                # drain: last LAG PV slots + final chain + e1 out-proj
                for t in range(st_n - LAG, st_n):
                    pv(nu - 1, t)
                evac(nu - 1)
                norm(nu - 1)
                while oq:
                    outproj(oq.pop(0))

    nc.compile()
    return nc


_NC_CACHE = {}


def _get_nc():
    key = (MM_DT, SCH_PER_UNIT)
    if key not in _NC_CACHE:
        _NC_CACHE[key] = build_mha(mm_dt=MM_DT, sch_per_unit=SCH_PER_UNIT)
    return _NC_CACHE[key]


def pack_x(xT, ktd=None, p=P):
    """[d, s] -> [p, (k s)] contiguous DMA layout."""
    d, s = xT.shape
    k = d // p
    return np.ascontiguousarray(xT.reshape(k, p, s).transpose(1, 0, 2).reshape(p, k * s))


def pack_xv(xT, st_n=None, p=P):
    """[d, s] -> [p, (t k c)] t-major layout for early V-proj tiles."""
    d, s = xT.shape
    k = d // p
    t = s // p
    return np.ascontiguousarray(
        xT.reshape(k, p, t, p).transpose(1, 2, 0, 3).reshape(p, t * k * p))


def pack_w(wT, p=P):
    """[d, jj] -> [p, (k jj)]."""
    d, jj = wT.shape
    k = d // p
    return np.ascontiguousarray(wT.reshape(k, p, jj).transpose(1, 0, 2).reshape(p, k * jj))


def build_in_maps(inputs, mm_dt=MM_DT):
    if mm_dt == "bf16":
        import ml_dtypes
        xdt = ml_dtypes.bfloat16
    else:
        xdt = np.float16

    q = np.asarray(inputs["query"], np.float32)
    k = np.asarray(inputs.get("key_", inputs.get("key")), np.float32)
    v = np.asarray(inputs["value"], np.float32)
    Wq = np.asarray(inputs["Wq"], np.float32)
    Wk = np.asarray(inputs["Wk"], np.float32)
    Wv = np.asarray(inputs["Wv"], np.float32)
    Wo = np.asarray(inputs["Wo"], np.float32)
    bq = np.asarray(inputs["bq"], np.float32)
    bk = np.asarray(inputs["bk"], np.float32)

    sc = np.float32(1.0 / np.sqrt(HD))

    qP = [pack_x(q[b].T.astype(xdt)) for b in range(B)]
    kP = [pack_x(k[b].T.astype(xdt)) for b in range(B)]
    vP = [pack_xv(v[b].T.astype(xdt)) for b in range(B)]
    WqT, WkT, WvT = Wq.T, Wk.T, Wv.T

    in_maps = []
    for core in range(N_CORES):
        b, g = divmod(core, GROUPS)
        sl = slice(g * JJ, (g + 1) * JJ)
        in_maps.append({
            "xq": qP[b],
            "xk": kP[b],
            "xv": vP[b],
            "wq": pack_w(np.ascontiguousarray(WqT[:, sl]).astype(xdt)),
            "wk": pack_w(np.ascontiguousarray(WkT[:, sl]).astype(xdt)),
            "wv": pack_w(np.ascontiguousarray(WvT[:, sl]).astype(xdt)),
            "wo": pack_w(np.ascontiguousarray(Wo[:, sl].T).astype(xdt)),
            "bqp": np.ascontiguousarray((bq[sl] * sc)[:, None]),
            "bkp": np.ascontiguousarray(bk[sl][:, None]),
        })
    return in_maps


def combine_outputs(results, inputs):
    Wo = np.asarray(inputs["Wo"], np.float32)
    bv = np.asarray(inputs["bv"], np.float32)
    bo = np.asarray(inputs["bo"], np.float32)
    const = bv @ Wo.T + bo  # exact host-side bias correction
    outp = np.empty((B, S, D), np.float32)
    for b in range(B):
        acc = results[b * GROUPS]["out"].astype(np.float32)
        for g in range(1, GROUPS):
            acc = acc + results[b * GROUPS + g]["out"].astype(np.float32)
        outp[b] = acc + const[None, :]
    return outp


def kernel(**inputs):
    import time
    from concourse.bass_utils import run_bass_kernel_spmd

    nc = _get_nc()
    in_maps = build_in_maps(inputs)
    last_err = None
    for attempt in range(3):
        try:
            res = run_bass_kernel_spmd(nc, in_maps, list(range(N_CORES)))
            return combine_outputs(res.results, inputs)
        except Exception as e:  # transient device wedge: retry
            last_err = e
            try:
                import jax
                import jax.numpy as jnp
                for dvc in jax.devices()[:N_CORES]:
                    jax.device_put(jnp.zeros((8, 8)), dvc).block_until_ready()
            except Exception:
                pass
            time.sleep(5.0 * (attempt + 1))
    raise last_err
